# revision 30
# baseline (speedup 1.0000x reference)
"""SPD BatchNorm (SPDNet-style) for Trainium2, 8 NeuronCores.

Problem: X (32, 64, 25, 2, 16, 16) fp32 batch of SPD matrices, running_mean (16,16).
  1. per-sample arithmetic mean over the flattened T*V*M axis (3200 mats/sample)
  2. Karcher mean (one Karcher-flow step) over the 32 per-sample means
  3. new_running_mean = geodesic(running_mean, mean, 0.1)
  4. X_centered = mean^{-1/2} @ X @ mean^{-1/2} for all 102400 matrices

Device split (data-parallel over N, 4 samples / core):
  pass 1: per-core sum-reduce of each sample's 3200 matrices -> [4,256] sums
          (bf16-rounded inputs via casting DMA, exact fp32 PSUM accumulation)
  host:   combine partials, Karcher mean + geodesic + invsqrtm via float64 eigh
          (37 eighs of 16x16 - negligible), build K = kron-style (256,256) operator
  pass 2: per-core congruence Y_b = Gi X_b Gi for 12800 matrices via
          Y_nat = X_vec^T K : bf16 PE-transposes of natural tiles to vec-layout,
          two accumulating bf16 matmuls against K halves -> natural-layout fp32
          output in PSUM, batched contiguous stores.
"""

import sys

if not any("trn_rl_repo" in p for p in sys.path):
    sys.path.insert(0, "/opt/trn_rl_repo")

import numpy as np

EPS = 1e-6
MOMENTUM = 0.1
N, T, V, M, n = 32, 64, 25, 2, 16
MATS_PER_SAMPLE = T * V * M            # 3200
N_CORES = 8
SAMPLES_PER_CORE = N // N_CORES        # 4
MATS_PER_CORE = SAMPLES_PER_CORE * MATS_PER_SAMPLE  # 12800
NVEC = n * n                           # 256
SUPER = 25                             # super-chunks of 512 matrices in pass 2
_cache = {}


# ---------------------------------------------------------------- host math
def _sym_apply(X, fn, clamp=True):
    w, Vv = np.linalg.eigh(X)
    if clamp:
        w = np.maximum(w, EPS)
    return np.einsum("...ij,...j,...kj->...ik", Vv, fn(w), Vv)


def _sqrtm(X):
    return _sym_apply(X, np.sqrt)


def _invsqrtm(X):
    return _sym_apply(X, lambda w: 1.0 / np.sqrt(w))


def _logm(X):
    return _sym_apply(X, np.log)


def _expm(X):
    return _sym_apply(X, np.exp, clamp=False)


def _powm(X, t):
    return _sym_apply(X, lambda w: w ** t)


def _bary_geom(Xm):
    G = Xm.mean(axis=0)
    Gs, Gi = _sqrtm(G), _invsqrtm(G)
    L = _logm(np.einsum("ij,bjk,kl->bil", Gi, Xm, Gi))
    return Gs @ _expm(L.mean(axis=0)) @ Gs


def _geodesic(A, B, t):
    As, Ai = _sqrtm(A), _invsqrtm(A)
    return As @ _powm(Ai @ B @ Ai, t) @ As


# ------------------------------------------------------------- bass modules
def _build_reduce():
    import concourse.tile as tile
    from concourse import bacc, mybir


    F32 = mybir.dt.float32
    BF16 = mybir.dt.bfloat16
    nc = bacc.Bacc()
    x = nc.dram_tensor(
        "x", [SAMPLES_PER_CORE, 128, SUPER * NVEC], F32, kind="ExternalInput"
    )
    # one [1,256] sum per sample (bf16-rounded inputs, fp32 PSUM accumulate)
    s_out = nc.dram_tensor("s", [SAMPLES_PER_CORE, NVEC], F32, kind="ExternalOutput")
    with tile.TileContext(nc) as tc:
        with (
            tc.tile_pool(name="const", bufs=1) as constp,
            tc.tile_pool(name="inp", bufs=3) as inp,
            tc.tile_pool(name="accp", bufs=2) as accp,
            tc.tile_pool(name="ps", bufs=2, space="PSUM") as psp,
        ):
            ones = constp.tile([128, 1], BF16)
            nc.gpsimd.memset(ones[:], 1.0)
            ones_f = constp.tile([128, 1], F32, tag="ones_f")
            nc.gpsimd.memset(ones_f[:], 1.0)
            HALF = 12 * NVEC  # strips 0..11 | 12..23 | 24
            W5 = 5 * NVEC
            for s in range(SAMPLES_PER_CORE):
                acc = accp.tile([1, NVEC], F32)
                if s < 2:
                    # PE path: casting SWDGE loads + bf16 ones-matmul chains
                    t = inp.tile([128, SUPER * NVEC], BF16)
                    nc.gpsimd.dma_start(out=t[:, 0:HALF], in_=x[s][:, 0:HALF])
                    nc.gpsimd.dma_start(
                        out=t[:, HALF:SUPER * NVEC], in_=x[s][:, HALF:SUPER * NVEC]
                    )
                    psA = psp.tile([1, 2 * NVEC], F32)
                    psC = psp.tile([1, 2 * NVEC], F32, tag="psC")
                    for m in range(6):
                        nc.tensor.matmul(
                            psA[:],
                            lhsT=ones[:],
                            rhs=t[:, 2 * m * NVEC:(2 * m + 2) * NVEC],
                            start=(m == 0),
                            stop=(m == 5),
                        )
                    for m in range(6):
                        o = HALF + 2 * m * NVEC
                        nc.tensor.matmul(
                            psC[:],
                            lhsT=ones[:],
                            rhs=t[:, o:o + 2 * NVEC],
                            start=(m == 0),
                            stop=(m == 5),
                        )
                    psB = psp.tile([1, NVEC], F32, tag="psB")
                    nc.tensor.matmul(
                        psB[:],
                        lhsT=ones[:],
                        rhs=t[:, 24 * NVEC:25 * NVEC],
                        start=True,
                        stop=True,
                    )
                    nc.vector.tensor_copy(out=acc[:], in_=psA[:, 0:NVEC])
                    nc.vector.tensor_add(acc[:], acc[:], psA[:, NVEC:2 * NVEC])
                    nc.vector.tensor_add(acc[:], acc[:], psC[:, 0:NVEC])
                    nc.vector.tensor_add(acc[:], acc[:], psC[:, NVEC:2 * NVEC])
                    nc.vector.tensor_add(acc[:], acc[:], psB[:])
                else:
                    # DVE path: HWDGE fp32 loads + exact fp32 tree adds
                    # (5 groups of 5 strips), then one ones-matmul over
                    # partitions
                    tf = inp.tile([128, SUPER * NVEC], F32, tag="tf")
                    nc.sync.dma_start(out=tf[:, 0:HALF], in_=x[s][:, 0:HALF])
                    nc.sync.dma_start(
                        out=tf[:, HALF:SUPER * NVEC], in_=x[s][:, HALF:SUPER * NVEC]
                    )
                    acc5 = accp.tile([128, W5], F32, tag="acc5")
                    nc.vector.tensor_add(acc5[:], tf[:, 0:W5], tf[:, W5:2 * W5])
                    for g in range(2, 5):
                        nc.vector.tensor_add(
                            acc5[:], acc5[:], tf[:, g * W5:(g + 1) * W5]
                        )
                    accp128 = accp.tile([128, NVEC], F32, tag="accp128")
                    nc.vector.tensor_add(
                        accp128[:], acc5[:, 0:NVEC], acc5[:, NVEC:2 * NVEC]
                    )
                    for g in range(2, 5):
                        nc.vector.tensor_add(
                            accp128[:], accp128[:], acc5[:, g * NVEC:(g + 1) * NVEC]
                        )
                    psD = psp.tile([1, NVEC], F32, tag="psD")
                    nc.tensor.matmul(
                        psD[:], lhsT=ones_f[:], rhs=accp128[:], start=True, stop=True
                    )
                    nc.vector.tensor_copy(out=acc[:], in_=psD[:])
                nc.sync.dma_start(out=s_out[s:s + 1], in_=acc[:])
    return nc


def _build_congruence():
    import concourse.tile as tile
    from concourse import bacc, mybir
    from concourse.masks import make_identity


    F32 = mybir.dt.float32
    BF16 = mybir.dt.bfloat16
    nc = bacc.Bacc()
    x = nc.dram_tensor("x", [SUPER, 128, 4 * NVEC], F32, kind="ExternalInput")
    k = nc.dram_tensor("k", [2, 128, NVEC], BF16, kind="ExternalInput")
    y = nc.dram_tensor("y", [SUPER, 128, 4, NVEC], F32, kind="ExternalOutput")
    with tile.TileContext(nc) as tc:
        with (
            tc.tile_pool(name="const", bufs=1) as constp,
            tc.tile_pool(name="inp", bufs=5) as inp,
            tc.tile_pool(name="t2sb", bufs=12) as t2sbp,
            tc.tile_pool(name="outsb", bufs=4) as outsbp,
            tc.tile_pool(name="t2ps", bufs=4, space="PSUM") as t2psp,
            tc.tile_pool(name="outps", bufs=4, space="PSUM") as outpsp,
        ):
            ident = constp.tile([128, 128], BF16)
            make_identity(nc, ident[:])
            kt = []
            for h in range(2):
                ktile = constp.tile([128, NVEC], BF16, tag=f"k{h}")
                nc.sync.dma_start(out=ktile[:], in_=k[h])
                kt.append(ktile)
            for cb in range(SUPER):
                # casting DMA (SWDGE): fp32 in DRAM -> bf16 in SBUF
                t1b = inp.tile([128, 4 * NVEC], BF16)
                nc.gpsimd.dma_start(out=t1b[:], in_=x[cb])
                o4 = outsbp.tile([128, 4 * NVEC], F32)
                for s in range(4):
                    ps_out = outpsp.tile([128, NVEC], F32)
                    for h in range(2):
                        c = 2 * s + h
                        t2p = t2psp.tile([128, 128], BF16)
                        nc.tensor.transpose(
                            t2p[:], t1b[:, c * 128:(c + 1) * 128], ident[:]
                        )
                        t2s = t2sbp.tile([128, 128], BF16)
                        if h == 0 or s == 0:
                            nc.vector.tensor_copy(out=t2s[:], in_=t2p[:])
                        else:
                            nc.scalar.copy(out=t2s[:], in_=t2p[:])
                        nc.tensor.matmul(
                            ps_out[:],
                            lhsT=t2s[:],
                            rhs=kt[h][:],
                            start=(h == 0),
                            stop=(h == 1),
                        )
                    osl = o4[:, s * NVEC:(s + 1) * NVEC]
                    if s % 2 == 0:
                        nc.vector.tensor_copy(out=osl, in_=ps_out[:])
                    else:
                        nc.scalar.copy(out=osl, in_=ps_out[:])
                nc.sync.dma_start(out=y[cb], in_=o4[:])
    return nc


def _get_modules():
    if "mods" not in _cache:
        nc_red = _build_reduce()
        nc_red.finalize()
        nc_con = _build_congruence()
        nc_con.finalize()
        _cache["mods"] = (nc_red, nc_con)
    return _cache["mods"]


# ----------------------------------------------------- profiling shims
def _install_profiling_shims():
    """This image lacks antenv.axon_hooks (the NTFF profile hook shim) and
    cannot upload artifacts; provide both so run_bass_kernel_spmd(trace=True)
    can return exec_time_ns from on-device NTFF profiles."""
    if _cache.get("shims"):
        return
    import contextlib
    import ctypes
    import types

    import antenv
    import concourse.bass_utils as bu

    lib = ctypes.CDLL("/opt/axon/libaxon_pjrt.so")
    hook = None
    if hasattr(lib, "axon_start_nrt_profile"):
        lib.axon_start_nrt_profile.argtypes = [
            ctypes.POINTER(ctypes.c_int64),
            ctypes.c_size_t,
        ]
        lib.axon_start_nrt_profile.restype = ctypes.c_int64
        lib.axon_stop_nrt_profile.argtypes = [ctypes.c_char_p]
        lib.axon_stop_nrt_profile.restype = ctypes.c_int64

        @contextlib.contextmanager
        def hook(output_dir, device_ids):
            import jax

            jax.devices()
            if device_ids:
                ids = (ctypes.c_int64 * len(device_ids))(*device_ids)
                rc = lib.axon_start_nrt_profile(ids, len(device_ids))
            else:
                rc = lib.axon_start_nrt_profile(None, 0)
            if rc != 0:
                raise RuntimeError(f"axon_start_nrt_profile rc={rc}")
            try:
                yield
            finally:
                n = lib.axon_stop_nrt_profile(str(output_dir).encode())
                print(f"profile: {n} file(s) written to {output_dir}", file=sys.stderr)

    mod = types.ModuleType("antenv.axon_hooks")
    mod.get_axon_ntff_profile_hook = lambda: hook
    mod.set_axon_ntff_profile_hook = lambda h: None
    sys.modules["antenv.axon_hooks"] = mod
    antenv.axon_hooks = mod

    bu.upload_artifacts = lambda tmpdir: f"local:{tmpdir}"
    _cache["shims"] = True


# ------------------------------------------------------------------ driver
def _run(X, running_mean, trace=False):
    from concourse.bass_utils import run_bass_kernel_spmd

    if trace:
        _install_profiling_shims()

    nc_red, nc_con = _get_modules()
    core_ids = list(range(N_CORES))

    Xf = np.ascontiguousarray(X, dtype=np.float32).reshape(N, MATS_PER_SAMPLE, NVEC)

    # ---- pass 1: per-sample partial sums
    in1 = []
    for c in core_ids:
        shard = Xf[c * SAMPLES_PER_CORE:(c + 1) * SAMPLES_PER_CORE]
        in1.append({"x": shard.reshape(SAMPLES_PER_CORE, 128, SUPER * NVEC)})
    r1 = run_bass_kernel_spmd(nc_red, in1, core_ids, trace=trace)

    sums = np.empty((N, NVEC), dtype=np.float64)
    for c in core_ids:
        sums[c * SAMPLES_PER_CORE:(c + 1) * SAMPLES_PER_CORE] = r1.results[c][
            "s"
        ].astype(np.float64)
    X_mean = (sums / MATS_PER_SAMPLE).reshape(N, n, n)
    X_mean = 0.5 * (X_mean + X_mean.transpose(0, 2, 1))

    # ---- host: Karcher mean, EMA, inverse sqrt, congruence operator
    mean = _bary_geom(X_mean)
    new_running_mean = _geodesic(np.asarray(running_mean, np.float64), mean, MOMENTUM)
    Gi = _invsqrtm(mean)
    # Y[i,j] = sum_{k,l} Gi[i,k] X[k,l] Gi[l,j]  ->  K[(i,j),(k,l)] (symmetric)
    import ml_dtypes

    K = np.einsum("ik,lj->ijkl", Gi, Gi).reshape(NVEC, NVEC)
    Kin = np.ascontiguousarray(K.reshape(2, 128, NVEC).astype(ml_dtypes.bfloat16))

    # ---- pass 2: congruence transform
    in2 = []
    for c in core_ids:
        shard = Xf[c * SAMPLES_PER_CORE:(c + 1) * SAMPLES_PER_CORE]
        in2.append({"x": shard.reshape(SUPER, 128, 4 * NVEC), "k": Kin})
    r2 = run_bass_kernel_spmd(nc_con, in2, core_ids, trace=trace)

    Y = np.empty((N, MATS_PER_SAMPLE, NVEC), dtype=np.float32)
    for c in core_ids:
        Y[c * SAMPLES_PER_CORE:(c + 1) * SAMPLES_PER_CORE] = (
            r2.results[c]["y"].reshape(SAMPLES_PER_CORE, MATS_PER_SAMPLE, NVEC)
        )

    X_centered = Y.reshape(N, T, V, M, n, n)
    return (
        (X_centered, new_running_mean.astype(np.float32)),
        (r1.exec_time_ns, r2.exec_time_ns),
    )


def kernel(X, running_mean):
    out, _ = _run(X, running_mean, trace=False)
    return out


# revision 31
# speedup vs baseline: 1.1410x; 1.1410x over previous
"""SPD BatchNorm (SPDNet-style) for Trainium2, 8 NeuronCores.

Problem: X (32, 64, 25, 2, 16, 16) fp32 batch of SPD matrices, running_mean (16,16).
  1. per-sample arithmetic mean over the flattened T*V*M axis (3200 mats/sample)
  2. Karcher mean (one Karcher-flow step) over the 32 per-sample means
  3. new_running_mean = geodesic(running_mean, mean, 0.1)
  4. X_centered = mean^{-1/2} @ X @ mean^{-1/2} for all 102400 matrices

Device split (data-parallel over N, 4 samples / core):
  pass 1: per-core sum-reduce of each sample's 3200 matrices -> [4,256] sums
          (bf16-rounded inputs via casting DMA, exact fp32 PSUM accumulation)
  host:   combine partials, Karcher mean + geodesic + invsqrtm via float64 eigh
          (37 eighs of 16x16 - negligible), build K = kron-style (256,256) operator
  pass 2: per-core congruence Y_b = Gi X_b Gi for 12800 matrices via
          Y_nat = X_vec^T K : bf16 PE-transposes of natural tiles to vec-layout,
          two accumulating bf16 matmuls against K halves -> natural-layout fp32
          output in PSUM, batched contiguous stores.
"""

import sys

if not any("trn_rl_repo" in p for p in sys.path):
    sys.path.insert(0, "/opt/trn_rl_repo")

import numpy as np

EPS = 1e-6
MOMENTUM = 0.1
N, T, V, M, n = 32, 64, 25, 2, 16
MATS_PER_SAMPLE = T * V * M            # 3200
N_CORES = 8
SAMPLES_PER_CORE = N // N_CORES        # 4
MATS_PER_CORE = SAMPLES_PER_CORE * MATS_PER_SAMPLE  # 12800
NVEC = n * n                           # 256
SUPER = 25                             # super-chunks of 512 matrices in pass 2
_cache = {}


# ---------------------------------------------------------------- host math
def _sym_apply(X, fn, clamp=True):
    w, Vv = np.linalg.eigh(X)
    if clamp:
        w = np.maximum(w, EPS)
    return np.einsum("...ij,...j,...kj->...ik", Vv, fn(w), Vv)


def _sqrtm(X):
    return _sym_apply(X, np.sqrt)


def _invsqrtm(X):
    return _sym_apply(X, lambda w: 1.0 / np.sqrt(w))


def _logm(X):
    return _sym_apply(X, np.log)


def _expm(X):
    return _sym_apply(X, np.exp, clamp=False)


def _powm(X, t):
    return _sym_apply(X, lambda w: w ** t)


def _bary_geom(Xm):
    G = Xm.mean(axis=0)
    Gs, Gi = _sqrtm(G), _invsqrtm(G)
    L = _logm(np.einsum("ij,bjk,kl->bil", Gi, Xm, Gi))
    return Gs @ _expm(L.mean(axis=0)) @ Gs


def _geodesic(A, B, t):
    As, Ai = _sqrtm(A), _invsqrtm(A)
    return As @ _powm(Ai @ B @ Ai, t) @ As


# ------------------------------------------------------------- bass modules
def _build_reduce():
    import concourse.tile as tile
    from concourse import bacc, mybir


    F32 = mybir.dt.float32
    BF16 = mybir.dt.bfloat16
    nc = bacc.Bacc()
    x = nc.dram_tensor(
        "x", [SAMPLES_PER_CORE, 128, SUPER * NVEC], F32, kind="ExternalInput"
    )
    # one [1,256] sum per sample (bf16-rounded inputs, fp32 PSUM accumulate)
    s_out = nc.dram_tensor("s", [SAMPLES_PER_CORE, NVEC], F32, kind="ExternalOutput")
    with tile.TileContext(nc) as tc:
        with (
            tc.tile_pool(name="const", bufs=1) as constp,
            tc.tile_pool(name="inp", bufs=3) as inp,
            tc.tile_pool(name="accp", bufs=2) as accp,
            tc.tile_pool(name="ps", bufs=2, space="PSUM") as psp,
        ):
            ones = constp.tile([128, 1], BF16)
            nc.gpsimd.memset(ones[:], 1.0)
            HALF = 12 * NVEC  # strips 0..11 | 12..23 | 24
            for s in range(SAMPLES_PER_CORE):
                # two half-loads per sample so the PE chain starts earlier
                # (casting DMA: fp32 DRAM -> bf16 SBUF)
                t = inp.tile([128, SUPER * NVEC], BF16)
                nc.gpsimd.dma_start(out=t[:, 0:HALF], in_=x[s][:, 0:HALF])
                nc.gpsimd.dma_start(
                    out=t[:, HALF:SUPER * NVEC], in_=x[s][:, HALF:SUPER * NVEC]
                )
                # sum over all 3200 matrices: 128 partitions via the ones
                # contraction; 25 strips via two parallel PSUM chains
                psA = psp.tile([1, 2 * NVEC], F32)
                psC = psp.tile([1, 2 * NVEC], F32, tag="psC")
                for m in range(6):
                    nc.tensor.matmul(
                        psA[:],
                        lhsT=ones[:],
                        rhs=t[:, 2 * m * NVEC:(2 * m + 2) * NVEC],
                        start=(m == 0),
                        stop=(m == 5),
                    )
                for m in range(6):
                    o = HALF + 2 * m * NVEC
                    nc.tensor.matmul(
                        psC[:],
                        lhsT=ones[:],
                        rhs=t[:, o:o + 2 * NVEC],
                        start=(m == 0),
                        stop=(m == 5),
                    )
                psB = psp.tile([1, NVEC], F32, tag="psB")
                nc.tensor.matmul(
                    psB[:],
                    lhsT=ones[:],
                    rhs=t[:, 24 * NVEC:25 * NVEC],
                    start=True,
                    stop=True,
                )
                acc = accp.tile([1, NVEC], F32)
                nc.vector.tensor_copy(out=acc[:], in_=psA[:, 0:NVEC])
                nc.vector.tensor_add(acc[:], acc[:], psA[:, NVEC:2 * NVEC])
                nc.vector.tensor_add(acc[:], acc[:], psC[:, 0:NVEC])
                nc.vector.tensor_add(acc[:], acc[:], psC[:, NVEC:2 * NVEC])
                nc.vector.tensor_add(acc[:], acc[:], psB[:])
                nc.sync.dma_start(out=s_out[s:s + 1], in_=acc[:])
    return nc


def _build_congruence():
    import concourse.tile as tile
    from concourse import bacc, mybir
    from concourse.masks import make_identity


    F32 = mybir.dt.float32
    BF16 = mybir.dt.bfloat16
    nc = bacc.Bacc()
    x = nc.dram_tensor("x", [SUPER, 128, 4 * NVEC], F32, kind="ExternalInput")
    k = nc.dram_tensor("k", [2, 128, NVEC], BF16, kind="ExternalInput")
    y = nc.dram_tensor("y", [SUPER, 128, 4, NVEC], F32, kind="ExternalOutput")
    with tile.TileContext(nc) as tc:
        with (
            tc.tile_pool(name="const", bufs=1) as constp,
            tc.tile_pool(name="inp", bufs=5) as inp,
            tc.tile_pool(name="t2sb", bufs=12) as t2sbp,
            tc.tile_pool(name="outsb", bufs=4) as outsbp,
            tc.tile_pool(name="t2ps", bufs=4, space="PSUM") as t2psp,
            tc.tile_pool(name="outps", bufs=4, space="PSUM") as outpsp,
        ):
            ident = constp.tile([128, 128], BF16)
            make_identity(nc, ident[:])
            kt = []
            for h in range(2):
                ktile = constp.tile([128, NVEC], BF16, tag=f"k{h}")
                nc.sync.dma_start(out=ktile[:], in_=k[h])
                kt.append(ktile)
            for cb in range(SUPER):
                # casting DMA (SWDGE): fp32 in DRAM -> bf16 in SBUF
                t1b = inp.tile([128, 4 * NVEC], BF16)
                nc.gpsimd.dma_start(out=t1b[:], in_=x[cb])
                o4 = outsbp.tile([128, 4 * NVEC], F32)
                for s in range(4):
                    ps_out = outpsp.tile([128, NVEC], F32)
                    for h in range(2):
                        c = 2 * s + h
                        t2p = t2psp.tile([128, 128], BF16)
                        nc.tensor.transpose(
                            t2p[:], t1b[:, c * 128:(c + 1) * 128], ident[:]
                        )
                        t2s = t2sbp.tile([128, 128], BF16)
                        if h == 0 or s == 0:
                            nc.vector.tensor_copy(out=t2s[:], in_=t2p[:])
                        else:
                            nc.scalar.copy(out=t2s[:], in_=t2p[:])
                        nc.tensor.matmul(
                            ps_out[:],
                            lhsT=t2s[:],
                            rhs=kt[h][:],
                            start=(h == 0),
                            stop=(h == 1),
                        )
                    osl = o4[:, s * NVEC:(s + 1) * NVEC]
                    if s % 2 == 0:
                        nc.vector.tensor_copy(out=osl, in_=ps_out[:])
                    else:
                        nc.scalar.copy(out=osl, in_=ps_out[:])
                nc.sync.dma_start(out=y[cb], in_=o4[:])
    return nc


def _get_modules():
    if "mods" not in _cache:
        nc_red = _build_reduce()
        nc_red.finalize()
        nc_con = _build_congruence()
        nc_con.finalize()
        _cache["mods"] = (nc_red, nc_con)
    return _cache["mods"]


# ----------------------------------------------------- profiling shims
def _install_profiling_shims():
    """This image lacks antenv.axon_hooks (the NTFF profile hook shim) and
    cannot upload artifacts; provide both so run_bass_kernel_spmd(trace=True)
    can return exec_time_ns from on-device NTFF profiles."""
    if _cache.get("shims"):
        return
    import contextlib
    import ctypes
    import types

    import antenv
    import concourse.bass_utils as bu

    lib = ctypes.CDLL("/opt/axon/libaxon_pjrt.so")
    hook = None
    if hasattr(lib, "axon_start_nrt_profile"):
        lib.axon_start_nrt_profile.argtypes = [
            ctypes.POINTER(ctypes.c_int64),
            ctypes.c_size_t,
        ]
        lib.axon_start_nrt_profile.restype = ctypes.c_int64
        lib.axon_stop_nrt_profile.argtypes = [ctypes.c_char_p]
        lib.axon_stop_nrt_profile.restype = ctypes.c_int64

        @contextlib.contextmanager
        def hook(output_dir, device_ids):
            import jax

            jax.devices()
            if device_ids:
                ids = (ctypes.c_int64 * len(device_ids))(*device_ids)
                rc = lib.axon_start_nrt_profile(ids, len(device_ids))
            else:
                rc = lib.axon_start_nrt_profile(None, 0)
            if rc != 0:
                raise RuntimeError(f"axon_start_nrt_profile rc={rc}")
            try:
                yield
            finally:
                n = lib.axon_stop_nrt_profile(str(output_dir).encode())
                print(f"profile: {n} file(s) written to {output_dir}", file=sys.stderr)

    mod = types.ModuleType("antenv.axon_hooks")
    mod.get_axon_ntff_profile_hook = lambda: hook
    mod.set_axon_ntff_profile_hook = lambda h: None
    sys.modules["antenv.axon_hooks"] = mod
    antenv.axon_hooks = mod

    bu.upload_artifacts = lambda tmpdir: f"local:{tmpdir}"
    _cache["shims"] = True


# ------------------------------------------------------------------ driver
def _run(X, running_mean, trace=False):
    from concourse.bass_utils import run_bass_kernel_spmd

    if trace:
        _install_profiling_shims()

    nc_red, nc_con = _get_modules()
    core_ids = list(range(N_CORES))

    Xf = np.ascontiguousarray(X, dtype=np.float32).reshape(N, MATS_PER_SAMPLE, NVEC)

    # ---- pass 1: per-sample partial sums
    in1 = []
    for c in core_ids:
        shard = Xf[c * SAMPLES_PER_CORE:(c + 1) * SAMPLES_PER_CORE]
        in1.append({"x": shard.reshape(SAMPLES_PER_CORE, 128, SUPER * NVEC)})
    r1 = run_bass_kernel_spmd(nc_red, in1, core_ids, trace=trace)

    sums = np.empty((N, NVEC), dtype=np.float64)
    for c in core_ids:
        sums[c * SAMPLES_PER_CORE:(c + 1) * SAMPLES_PER_CORE] = r1.results[c][
            "s"
        ].astype(np.float64)
    X_mean = (sums / MATS_PER_SAMPLE).reshape(N, n, n)
    X_mean = 0.5 * (X_mean + X_mean.transpose(0, 2, 1))

    # ---- host: Karcher mean, EMA, inverse sqrt, congruence operator
    mean = _bary_geom(X_mean)
    new_running_mean = _geodesic(np.asarray(running_mean, np.float64), mean, MOMENTUM)
    Gi = _invsqrtm(mean)
    # Y[i,j] = sum_{k,l} Gi[i,k] X[k,l] Gi[l,j]  ->  K[(i,j),(k,l)] (symmetric)
    import ml_dtypes

    K = np.einsum("ik,lj->ijkl", Gi, Gi).reshape(NVEC, NVEC)
    Kin = np.ascontiguousarray(K.reshape(2, 128, NVEC).astype(ml_dtypes.bfloat16))

    # ---- pass 2: congruence transform
    in2 = []
    for c in core_ids:
        shard = Xf[c * SAMPLES_PER_CORE:(c + 1) * SAMPLES_PER_CORE]
        in2.append({"x": shard.reshape(SUPER, 128, 4 * NVEC), "k": Kin})
    r2 = run_bass_kernel_spmd(nc_con, in2, core_ids, trace=trace)

    Y = np.empty((N, MATS_PER_SAMPLE, NVEC), dtype=np.float32)
    for c in core_ids:
        Y[c * SAMPLES_PER_CORE:(c + 1) * SAMPLES_PER_CORE] = (
            r2.results[c]["y"].reshape(SAMPLES_PER_CORE, MATS_PER_SAMPLE, NVEC)
        )

    X_centered = Y.reshape(N, T, V, M, n, n)
    return (
        (X_centered, new_running_mean.astype(np.float32)),
        (r1.exec_time_ns, r2.exec_time_ns),
    )


def kernel(X, running_mean):
    out, _ = _run(X, running_mean, trace=False)
    return out
